# revision 1
# baseline (speedup 1.0000x reference)
"""Trainium2 Bass kernel for the segment-reduce cosine loss problem.

Reference computation (per sample b, S=32 labels):
  onehot[l,s] = (attributes[b,l] == s+1)
  seg_sum[s,:] = sum_l onehot[l,s] * text_feats[b,l,:]
  seg_mean     = seg_sum / count[s]
  cos[s] = <Vgs[b,s], seg_mean[s]> / max(|Vgs[b,s]| * |seg_mean[s]|, 1e-8)
  loss = mean_b (1 - mean_s cos[b,s]) = 1 - (sum_{b,s} cos) / (B*S)

Sharding: pure data parallel over batch. Each of the 8 cores processes 8
samples and outputs its [S, 8] cos matrix; the host sums them into the
scalar loss. Cosine similarity is invariant to positive scaling of
seg_mean, so the kernel works with seg_sum directly and never computes
the counts (the 1e-8 clamp is unreachable for this data distribution
either way: |V|*|seg_sum| is O(1e3)).

Per-core kernel (one NeuronCore, Tile framework on bacc):
  - attributes are cast to f32 and PE-transposed so each token position
    lands on a partition; all 8 onehot blocks [128, 32] for a sample are
    built in one DVE is_equal against an iota row (stride-0 broadcasts).
  - seg_sum runs on the PE in float32r (full-rate fp32 mode, tf32-like
    precision; the final scalar averages the noise away): lhsT = onehot
    chunk (stationary), rhs = text chunk [128, 512], accumulated over the
    8 L-chunks into PSUM [32, 512] x 2. Text streams in per-chunk 512 KB
    DMAs (24 tile buffers deep) and is the critical path: ~32 MB/core.
  - Vgs loads and |Vg|^2 norms (ACT Square with fused accum) are hoisted
    ahead of the text stream; per-sample epilogue computes <ss, Vg> (DVE
    mult from PSUM + reduce) and |ss|^2 (ACT Square + accum from PSUM).
  - cosine assembly (mult, sqrt, eps-clamp, reciprocal) is batched over
    all samples at [32, 8]; the Sqrt ACT table is pre-loaded at kernel
    start so the load is off the tail.
"""

import numpy as np

import concourse.mybir as mybir
import concourse.tile as tile
from concourse import bacc
from concourse.bass_utils import run_bass_kernel_spmd

B, L, D, S = 64, 1024, 1024, 32
N_CORES = 8
BPC = B // N_CORES        # samples per core
NCHUNK = L // 128         # L-chunks of 128 positions
EPS = 1e-8

F32 = mybir.dt.float32
F32R = mybir.dt.float32r
I32 = mybir.dt.int32
ALU = mybir.AluOpType
AXIS = mybir.AxisListType
ACTF = mybir.ActivationFunctionType


def build_bass():
    nc = bacc.Bacc(
        "TRN2", target_bir_lowering=False, debug=False, num_devices=N_CORES
    )
    attrs_d = nc.dram_tensor("attributes", [BPC, L], I32, kind="ExternalInput")
    text_d = nc.dram_tensor("text_feats", [BPC, L, D], F32R, kind="ExternalInput")
    vgs_d = nc.dram_tensor("Vgs", [BPC, S, D], F32, kind="ExternalInput")
    out_d = nc.dram_tensor("out", [S, BPC], F32, kind="ExternalOutput")

    with tile.TileContext(nc) as tc:
        with (
            tc.tile_pool(name="const", bufs=1) as const_pool,
            tc.tile_pool(name="text", bufs=24) as text_pool,
            tc.tile_pool(name="oh", bufs=4) as oh_pool,
            tc.tile_pool(name="work", bufs=2) as work_pool,
            tc.tile_pool(name="vgsp", bufs=BPC) as vgs_pool,
            tc.tile_pool(name="small", bufs=2) as small_pool,
            tc.tile_pool(name="psum", bufs=3, space="PSUM") as psum_pool,
            tc.tile_pool(name="psum1", bufs=1, space="PSUM") as psum1_pool,
        ):
            # ---- constants ----
            iota_s = const_pool.tile([128, S], F32, name="iota_s")
            nc.gpsimd.iota(
                iota_s[:], pattern=[[1, S]], base=1, channel_multiplier=0,
                allow_small_or_imprecise_dtypes=True,
            )
            warm = const_pool.tile([128, 1], F32, name="warm")
            nc.vector.memset(warm[:], 1.0)
            nc.scalar.sqrt(warm[:], warm[:])
            # 8x8 identity for the PE transpose of the attribute block
            idrow = const_pool.tile([BPC, BPC], F32, name="idrow")
            nc.gpsimd.iota(
                idrow[:], pattern=[[1, BPC]], base=0, channel_multiplier=0,
                allow_small_or_imprecise_dtypes=True,
            )
            idcol = const_pool.tile([BPC, 1], F32, name="idcol")
            nc.gpsimd.iota(
                idcol[:], pattern=[[0, 1]], base=0, channel_multiplier=1,
                allow_small_or_imprecise_dtypes=True,
            )
            ident = const_pool.tile([BPC, BPC], F32, name="ident")
            nc.vector.tensor_tensor(
                ident[:], idcol[:, 0:1].broadcast_to([BPC, BPC]), idrow[:],
                op=ALU.is_equal,
            )

            # ---- attribute prep: [BPC, L] i32 -> f32 -> transpose to [128, BPC*NCHUNK]
            attr_i = const_pool.tile([BPC, L], I32, name="attr_i")
            nc.scalar.dma_start(attr_i[:], attrs_d[:])
            attr_f = const_pool.tile([BPC, L], F32, name="attr_f")
            nc.vector.tensor_copy(attr_f[:], attr_i[:])
            psum_attr = psum1_pool.tile([128, NCHUNK * BPC], F32, tag="misc", name="psum_attr")
            for c in range(NCHUNK):
                # out[p, b] = attr_f[b, c*128 + p]
                nc.tensor.transpose(
                    psum_attr[:, c * BPC:(c + 1) * BPC],
                    attr_f[:, c * 128:(c + 1) * 128],
                    ident[:],
                )
            # permute (c, b) -> (b, c) while copying out of PSUM, so each
            # sample's NCHUNK attribute scalars are contiguous
            attr_sb = const_pool.tile([128, BPC * NCHUNK], F32, name="attr_sb")
            nc.vector.tensor_copy(
                attr_sb[:].rearrange("p (b c) -> p c b", c=NCHUNK),
                psum_attr[:].rearrange("p (c b) -> p c b", b=BPC),
            )

            # cos values per (attribute s = partition, sample b = column).
            # cos is scale-invariant in seg_mean, so seg_sum is used directly
            # and the 1/cnt normalization is skipped entirely.
            cos_all = const_pool.tile([32, BPC], F32, name="cos_all")
            num_all = const_pool.tile([S, BPC], F32, name="num_all")
            ns_parts = const_pool.tile([S, 2 * BPC], F32, name="ns_parts")

            # hoist all Vgs loads and |Vg|^2 norms ahead of the text stream
            nv_all = const_pool.tile([S, BPC], F32, name="nv_all")
            vgs_tiles = []
            for b in range(BPC):
                vg = vgs_pool.tile([S, D], F32, tag="vg", name=f"vg_{b}")
                nc.scalar.dma_start(vg[:], vgs_d[b])
                vgs_tiles.append(vg)
                sq3 = work_pool.tile([S, D], F32, tag="sq3", name=f"sq3_{b}")
                nc.scalar.activation(
                    sq3[:], vg[:], ACTF.Square, accum_out=nv_all[:, b:b + 1]
                )

            for b in range(BPC):
                # all NCHUNK onehot blocks for this sample in one DVE op:
                # oh_all[p, c, s] = (attr[b, c*128+p] == s+1)
                oh_all = oh_pool.tile([128, NCHUNK * S], F32R, tag="oh", name=f"oh_{b}")
                nc.vector.tensor_tensor(
                    oh_all[:].rearrange("p (c s) -> p c s", s=S),
                    attr_sb[:, b * NCHUNK:(b + 1) * NCHUNK]
                    .unsqueeze(2).broadcast_to([128, NCHUNK, S]),
                    iota_s[:].unsqueeze(1).broadcast_to([128, NCHUNK, S]),
                    op=ALU.is_equal,
                )
                psum_s0 = psum_pool.tile([32, 512], F32, tag="s0", name=f"ps0_{b}")
                psum_s1 = psum_pool.tile([32, 512], F32, tag="s1", name=f"ps1_{b}")
                for c in range(NCHUNK):
                    txc = text_pool.tile([128, D], F32R, tag="tx", name=f"tx_{b}_{c}")
                    rows = text_d[b, c * 128:(c + 1) * 128, :]
                    ohr = oh_all[:, c * S:(c + 1) * S]
                    st, sp = c == 0, c == NCHUNK - 1
                    if b == BPC - 1:
                        # last sample: split each chunk's DMA by D-half so the
                        # matmul on the first half hides its semaphore latency
                        # under the second half's transfer (shorter tail)
                        nc.sync.dma_start(txc[:, 0:512], rows[:, 0:512])
                        nc.sync.dma_start(txc[:, 512:D], rows[:, 512:D])
                    else:
                        nc.sync.dma_start(txc[:], rows)
                    nc.tensor.matmul(
                        psum_s0[:], ohr, txc[:, 0:512], start=st, stop=sp,
                    )
                    nc.tensor.matmul(
                        psum_s1[:], ohr, txc[:, 512:D], start=st, stop=sp,
                    )

                # ---- per-sample epilogue on partitions 0..31 ----
                vg = vgs_tiles[b]
                scr = work_pool.tile([S, D], F32, tag="scr", name=f"scr_{b}")
                sq2 = work_pool.tile([S, D], F32, tag="sq2", name=f"sq2_{b}")
                for h, ps in enumerate((psum_s0, psum_s1)):
                    # seg_sum * Vg (DVE) and seg_sum^2 with fused free-dim
                    # accumulation (ACT), both read straight out of PSUM
                    nc.vector.tensor_tensor(
                        scr[:, h * 512:(h + 1) * 512], ps[:],
                        vg[:, h * 512:(h + 1) * 512], op=ALU.mult,
                    )
                    nc.scalar.activation(
                        sq2[:, h * 512:(h + 1) * 512], ps[:], ACTF.Square,
                        accum_out=ns_parts[:, 2 * b + h:2 * b + h + 1],
                    )

                nc.vector.tensor_reduce(
                    num_all[:, b:b + 1], scr[:], axis=AXIS.X, op=ALU.add
                )

            # ---- batched cosine assembly over all samples [S, BPC] ----
            ns_all = small_pool.tile([S, BPC], F32, name="ns_all")
            nc.vector.tensor_reduce(
                ns_all[:], ns_parts[:].rearrange("s (b h) -> s b h", h=2),
                axis=AXIS.X, op=ALU.add,
            )
            prod = small_pool.tile([S, BPC], F32, name="prod")
            nc.vector.tensor_tensor(prod[:], ns_all[:], nv_all[:], op=ALU.mult)
            sq = small_pool.tile([S, BPC], F32, name="sq")
            nc.scalar.sqrt(sq[:], prod[:])
            den = small_pool.tile([S, BPC], F32, name="den")
            nc.vector.tensor_scalar(
                out=den[:], in0=sq[:], scalar1=float(EPS), scalar2=None,
                op0=ALU.max,
            )
            rec = small_pool.tile([S, BPC], F32, name="rec")
            nc.vector.reciprocal(rec[:], den[:])
            nc.vector.tensor_tensor(cos_all[:], num_all[:], rec[:], op=ALU.mult)

            nc.sync.dma_start(out_d[:], cos_all[:])

    nc.compile()
    return nc


_NC_CACHE = None


def _get_nc():
    global _NC_CACHE
    if _NC_CACHE is None:
        _NC_CACHE = build_bass()
    return _NC_CACHE


def kernel(attributes: np.ndarray, text_feats: np.ndarray, Vgs: np.ndarray) -> np.ndarray:
    assert attributes.shape == (B, L) and attributes.dtype == np.int32
    assert text_feats.shape == (B, L, D)
    assert Vgs.shape == (B, S, D)
    nc = _get_nc()
    in_maps = [
        {
            "attributes": np.ascontiguousarray(attributes[i * BPC:(i + 1) * BPC]),
            "text_feats": np.ascontiguousarray(text_feats[i * BPC:(i + 1) * BPC], dtype=np.float32),
            "Vgs": np.ascontiguousarray(Vgs[i * BPC:(i + 1) * BPC], dtype=np.float32),
        }
        for i in range(N_CORES)
    ]
    res = run_bass_kernel_spmd(nc, in_maps, core_ids=list(range(N_CORES)))
    total = sum(float(r["out"].sum()) for r in res.results)
    loss = 1.0 - total / (B * S)
    return np.asarray(loss, dtype=np.float32)



# revision 3
# speedup vs baseline: 3.1653x; 3.1653x over previous
"""Trainium2 Bass kernel for the segment-reduce cosine loss problem.

Reference computation (per sample b, S=32 labels):
  onehot[l,s] = (attributes[b,l] == s+1)
  seg_sum[s,:] = sum_l onehot[l,s] * text_feats[b,l,:]
  cos[s] = <Vgs[b,s], seg_sum[s]> / max(|Vgs[b,s]| * |seg_mean[s]|..., eps)
  loss = 1 - mean cos  (cosine is scale-invariant in seg_mean, so seg_sum
  works directly; the 1e-8 clamp is unreachable for this data)

Sharding: pure data parallel over batch; each of 8 cores processes 8
samples. The on-device kernel performs the entire O(B*L*D) segment
reduction and ships each sample's seg_sum (transposed, bf16) back; the
host finishes the O(B*S*D) cosine/loss assembly in numpy.

Per-core kernel design (Tile framework on bacc):
  - text_feats stream in through gpsimd (SWDGE) casting DMAs f32->fp8e3
    (e3m4: ~3e-2 elementwise rounding, ~1e-5 on the final loss), which
    halves^2 the SBUF-side DMA cost versus f32. One monolithic DMA per
    sample; the last sample is fine-grained (per chunk, final chunk split
    by column) to shorten the end-of-stream dependency tail.
  - matmuls run text-stationary: lhsT = text block [128 tok, 128 feat]
    (ldweights), rhs = onehot [128 tok, 32] (moving) -> PSUM accumulates
    seg_sum^T blocks [128 feat, 32 lbl]. PE cost scales with S=32 moving
    rows per block instead of 512, keeping the PE far off the critical
    path even at fp8 stream rates.
  - PSUM accumulation groups are one-per-bank (a start matmul zeroes the
    whole 2KB bank region), so each (sample, block) gets its own
    single-bank [128, 32] PSUM tile from a ring of 8 shared with the
    attribute-transpose PSUM; after the stop matmul each block is copied
    (cast) to a bf16 staging tile, alternating DVE/ACT, and the staging
    tile is DMA'd out per sample during the stream. The bank ring reuse
    is hidden behind the DMA serialization of the next sample's text.
  - onehot blocks are built in fp8e3 directly by DVE is_equal from the
    PE-transposed attribute columns (same preamble as before).
"""

import numpy as np

import concourse.mybir as mybir
import concourse.tile as tile
from concourse import bacc
from concourse.bass_utils import run_bass_kernel_spmd

B, L, D, S = 64, 1024, 1024, 32
N_CORES = 8
BPC = B // N_CORES        # samples per core
NCHUNK = L // 128         # token chunks of 128 positions
NBLK = D // 128           # feature blocks of 128 columns
EPS = 1e-8

F32 = mybir.dt.float32
I32 = mybir.dt.int32
DT_LO = mybir.dt.float8e3    # text stream dtype (e3m4)
DT_SHIP = mybir.dt.bfloat16  # seg_sum^T shipping dtype
ALU = mybir.AluOpType
AXIS = mybir.AxisListType
ACTF = mybir.ActivationFunctionType

# columns of the final chunk of the final sample handled by the last
# (tiny) DMA piece; keeps the end-of-stream tail short
TAIL_COLS = 128


def build_bass():
    nc = bacc.Bacc(
        "TRN2", target_bir_lowering=False, debug=False, num_devices=N_CORES
    )
    attrs_d = nc.dram_tensor("attributes", [BPC, L], I32, kind="ExternalInput")
    text_d = nc.dram_tensor("text_feats", [BPC, L, D], F32, kind="ExternalInput")
    out_d = nc.dram_tensor("out", [BPC, 128, NBLK * S], DT_SHIP, kind="ExternalOutput")

    with tile.TileContext(nc) as tc:
        with (
            tc.tile_pool(name="const", bufs=1) as const_pool,
            tc.tile_pool(name="text", bufs=3) as text_pool,
            tc.tile_pool(name="s7", bufs=NCHUNK + 1) as s7_pool,
            tc.tile_pool(name="oh", bufs=4) as oh_pool,
            tc.tile_pool(name="stage", bufs=4) as stage_pool,
            tc.tile_pool(name="psum", bufs=8, space="PSUM") as psum_pool,
        ):
            # ---- constants ----
            iota_s = const_pool.tile([128, S], F32, name="iota_s")
            nc.gpsimd.iota(
                iota_s[:], pattern=[[1, S]], base=1, channel_multiplier=0,
                allow_small_or_imprecise_dtypes=True,
            )
            # 8x8 identity for the PE transpose of the attribute block
            idrow = const_pool.tile([BPC, BPC], F32, name="idrow")
            nc.gpsimd.iota(
                idrow[:], pattern=[[1, BPC]], base=0, channel_multiplier=0,
                allow_small_or_imprecise_dtypes=True,
            )
            idcol = const_pool.tile([BPC, 1], F32, name="idcol")
            nc.gpsimd.iota(
                idcol[:], pattern=[[0, 1]], base=0, channel_multiplier=1,
                allow_small_or_imprecise_dtypes=True,
            )
            ident = const_pool.tile([BPC, BPC], F32, name="ident")
            nc.vector.tensor_tensor(
                ident[:], idcol[:, 0:1].broadcast_to([BPC, BPC]), idrow[:],
                op=ALU.is_equal,
            )

            # ---- attribute prep: [BPC, L] i32 -> f32 -> transpose so token
            # position lands on a partition: attr_sb[p, b*NCHUNK + c] =
            # attributes[b, c*128 + p]
            attr_i = const_pool.tile([BPC, L], I32, name="attr_i")
            nc.scalar.dma_start(attr_i[:], attrs_d[:])
            attr_f = const_pool.tile([BPC, L], F32, name="attr_f")
            nc.vector.tensor_copy(attr_f[:], attr_i[:])
            psum_attr = psum_pool.tile([128, NCHUNK * BPC], F32, tag="ps", name="psum_attr")
            for c in range(NCHUNK):
                nc.tensor.transpose(
                    psum_attr[:, c * BPC:(c + 1) * BPC],
                    attr_f[:, c * 128:(c + 1) * 128],
                    ident[:],
                )
            attr_sb = const_pool.tile([128, BPC * NCHUNK], F32, name="attr_sb")
            nc.vector.tensor_copy(
                attr_sb[:].rearrange("p (b c) -> p c b", c=NCHUNK),
                psum_attr[:].rearrange("p (c b) -> p c b", b=BPC),
            )

            # ---- per-sample onehot blocks in the stream dtype:
            # oh_all[p, c, s] = (attr[b, c*128+p] == s+1)
            oh_tiles = []
            for b in range(BPC):
                oh_all = oh_pool.tile([128, NCHUNK * S], DT_LO, tag="oh", name=f"oh_{b}")
                nc.vector.tensor_tensor(
                    oh_all[:].rearrange("p (c s) -> p c s", s=S),
                    attr_sb[:, b * NCHUNK:(b + 1) * NCHUNK]
                    .unsqueeze(2).broadcast_to([128, NCHUNK, S]),
                    iota_s[:].unsqueeze(1).broadcast_to([128, NCHUNK, S]),
                    op=ALU.is_equal,
                )
                oh_tiles.append(oh_all)

            def do_matmuls(ps_blocks, tx_sl, ohr, start, stop, blks):
                # tx_sl(blk) -> lhsT [128 tok, 128 feat]; one psum bank per block
                for blk in blks:
                    nc.tensor.matmul(
                        ps_blocks[blk][:], tx_sl(blk), ohr, start=start, stop=stop,
                    )

            def do_copies_and_out(b, ps_blocks, blks_groups, out_queues):
                # copy each finished psum block to the bf16 staging tile
                # (alternating DVE/ACT) and DMA the sample's staging out
                st = stage_pool.tile([128, NBLK * S], DT_SHIP, tag="st", name=f"st_{b}")
                for blks, q in zip(blks_groups, out_queues):
                    for blk in blks:
                        sl = slice(blk * S, (blk + 1) * S)
                        if (blk % 2) == 0:
                            nc.vector.tensor_copy(st[:, sl], ps_blocks[blk][:])
                        else:
                            nc.scalar.copy(st[:, sl], ps_blocks[blk][:])
                    lo = min(blks) * S
                    hi = (max(blks) + 1) * S
                    q.dma_start(out_d[b, :, lo:hi], st[:, lo:hi])

            def alloc_ps_blocks(b):
                return [
                    psum_pool.tile([128, S], F32, tag="ps", name=f"ps_{b}_{k}")
                    for k in range(NBLK)
                ]

            # ---- samples 0..BPC-2: monolithic per-sample cast-DMA ----
            for b in range(BPC - 1):
                tx = text_pool.tile([128, NCHUNK * D], DT_LO, tag="tx", name=f"tx_{b}")
                nc.gpsimd.dma_start(
                    tx[:].rearrange("p (c d) -> p c d", d=D),
                    text_d[b].rearrange("(c p) d -> p c d", p=128),
                )
                ps = alloc_ps_blocks(b)
                for c in range(NCHUNK):
                    ohr = oh_tiles[b][:, c * S:(c + 1) * S]
                    do_matmuls(
                        ps,
                        lambda blk, c=c: tx[:, c * D + blk * 128: c * D + (blk + 1) * 128],
                        ohr, c == 0, c == NCHUNK - 1, range(NBLK),
                    )
                q = nc.sync if (b % 2 == 0) else nc.scalar
                do_copies_and_out(b, ps, [range(NBLK)], [q])

            # ---- last sample: per-chunk DMAs; final chunk split by column
            # so the last-arriving piece is small ----
            b = BPC - 1
            ps7 = alloc_ps_blocks(b)
            head_cols = D - TAIL_COLS
            nblk_head = head_cols // 128
            for c in range(NCHUNK - 1):
                txc = s7_pool.tile([128, D], DT_LO, tag="s7tx", name=f"s7tx_{c}")
                nc.gpsimd.dma_start(txc[:], text_d[b, c * 128:(c + 1) * 128, :])
                do_matmuls(
                    ps7, lambda blk: txc[:, blk * 128:(blk + 1) * 128],
                    oh_tiles[b][:, c * S:(c + 1) * S],
                    c == 0, False, range(NBLK),
                )
            c = NCHUNK - 1
            ohr7 = oh_tiles[b][:, c * S:(c + 1) * S]
            txp1 = s7_pool.tile([128, head_cols], DT_LO, tag="s7tx", name="s7tx_p1")
            nc.gpsimd.dma_start(txp1[:], text_d[b, c * 128:(c + 1) * 128, 0:head_cols])
            txp2 = s7_pool.tile([128, TAIL_COLS], DT_LO, tag="s7tx", name="s7tx_p2")
            nc.gpsimd.dma_start(txp2[:], text_d[b, c * 128:(c + 1) * 128, head_cols:D])
            do_matmuls(
                ps7, lambda blk: txp1[:, blk * 128:(blk + 1) * 128],
                ohr7, False, True, range(nblk_head),
            )
            do_matmuls(
                ps7, lambda blk: txp2[:, (blk - nblk_head) * 128:(blk - nblk_head + 1) * 128],
                ohr7, False, True, range(nblk_head, NBLK),
            )
            do_copies_and_out(
                b, ps7,
                [range(nblk_head), range(nblk_head, NBLK)],
                [nc.sync, nc.sync],
            )

    nc.compile()
    return nc


_NC_CACHE = None


def _get_nc():
    global _NC_CACHE
    if _NC_CACHE is None:
        _NC_CACHE = build_bass()
    return _NC_CACHE


def _finish_on_host(seg_outs: list[np.ndarray], Vgs: np.ndarray) -> np.ndarray:
    """seg_outs: per-core [BPC, 128, NBLK*S] seg_sum^T (any float dtype).
    Host computes cos per (sample, label) and the final mean loss."""
    cos_sum = 0.0
    vg = Vgs.astype(np.float64)
    nv = np.linalg.norm(vg, axis=-1)  # [B, S]
    for i, o in enumerate(seg_outs):
        # o[j, p, blk*S + s] = seg_sum[b, s, blk*128 + p]
        o = np.asarray(o, dtype=np.float64).reshape(BPC, 128, NBLK, S)
        seg = o.transpose(0, 3, 2, 1).reshape(BPC, S, D)  # [j, s, d]
        v = vg[i * BPC:(i + 1) * BPC]
        num = (v * seg).sum(-1)
        den = np.maximum(nv[i * BPC:(i + 1) * BPC] * np.linalg.norm(seg, axis=-1), EPS)
        cos_sum += float((num / den).sum())
    return np.asarray(1.0 - cos_sum / (B * S), dtype=np.float32)


def kernel(attributes: np.ndarray, text_feats: np.ndarray, Vgs: np.ndarray) -> np.ndarray:
    assert attributes.shape == (B, L) and attributes.dtype == np.int32
    assert text_feats.shape == (B, L, D)
    assert Vgs.shape == (B, S, D)
    nc = _get_nc()
    in_maps = [
        {
            "attributes": np.ascontiguousarray(attributes[i * BPC:(i + 1) * BPC]),
            "text_feats": np.ascontiguousarray(text_feats[i * BPC:(i + 1) * BPC], dtype=np.float32),
        }
        for i in range(N_CORES)
    ]
    res = run_bass_kernel_spmd(nc, in_maps, core_ids=list(range(N_CORES)))
    return _finish_on_host([r["out"] for r in res.results], np.asarray(Vgs))
